# revision 6
# baseline (speedup 1.0000x reference)
"""Trainium2 Bass kernel for nn_CICDM (Choquet-integral cognitive-diagnosis model).

Computation (see reference):
  sel = sigmoid(emb[stu_id])                       # [B, 30]
  x_k = sel[:, q_idx[:, k]]  k=0,1,2               # [B, N]
  C   = Choquet integral of (x0,x1,x2) against fuzzy measure FM(fm_vars)
  out = sigmoid(relu(relu(C@w1.T+b1)@w2.T+b2)@w3.T+b3)

Key reformulation: the sorted-difference Choquet integral equals its Mobius
form  C = sum_S m(S) * min_{i in S} x_i  which, with hinge algebra
(min(a,b) = a - relu(a-b)), becomes

  C = c0*x0 + c1*x1 + c2*x2 + a01*r01 + a02*r02 + a12*r12 + au*u
  r01 = relu(x0-x1), r02 = relu(x0-x2), r12 = relu(x1-x2), u = relu((x0-x2)-r01)

with per-exercise constants c*/a* derived from fm_vars on the host.  The
per-exercise gathers x_k and differences are one-hot / +-1 matmuls on the
tensor engine (K=30), the per-exercise scaling is a diagonal-matmul
accumulation into PSUM, and the hinges run on the scalar/vector/gpsimd
engines.

Distribution: data-parallel over the batch: 8 cores x 512 rows. Everything is
computed in transposed layout (exercises on partitions, batch on free dim) so
per-exercise coefficients are per-partition scalars.
"""

import numpy as np

B = 4096
NCORES = 8
BL = B // NCORES          # 512 local batch
KN = 30
NOUT = 1024
NT = NOUT // 128          # 8 exercise tiles
P = 128
NG = BL // P              # 4 gather groups per core
S_N = 100000
N_WARM = 14               # PE warm-up matmuls issued during the DMA phase

_PROG_CACHE = {}


def _np_dt(name):
    if name == "bfloat16":
        import ml_dtypes
        return np.dtype(ml_dtypes.bfloat16)
    return np.dtype(np.float32)


def _host_prep(q_idx, fm_vars, w1, b1, w2, b2, w3, b3, mm_dtype_name):
    """Derive all per-exercise constants + weight layouts on the host."""
    mmnp = _np_dt(mm_dtype_name)
    q = np.asarray(q_idx).astype(np.int64)          # [N, 3]
    fm = np.asarray(fm_vars, dtype=np.float32)

    chi = np.abs(fm)
    f0, f1 = chi[0], chi[1]
    f2 = np.maximum(f0, f1) + chi[2]
    f3 = chi[3]
    f4 = np.maximum(f3, f0) + chi[4]
    f5 = np.maximum(f3, f1) + chi[5]
    FM = np.minimum(np.stack([f0, f1, f2, f3, f4, f5, np.ones_like(f0)], 0), 1.0)
    F0, F1, F2, F3, F4, F5, F6 = FM.astype(np.float64)
    m0, m1, m3 = F0, F1, F3
    m2 = F2 - F0 - F1
    m4 = F4 - F0 - F3
    m5 = F5 - F1 - F3
    m6 = F6 - F2 - F4 - F5 + F0 + F1 + F3
    c0 = (m0 + m2 + m4 + m6).astype(np.float32)
    c1 = (m1 + m5).astype(np.float32)
    c2 = m3.astype(np.float32)
    a01 = (-(m2 + m6)).astype(np.float32)
    a02 = (-m4).astype(np.float32)
    a12 = (-m5).astype(np.float32)
    au = (-m6).astype(np.float32)

    # gcat: per tile t, 4 lhsT planes [30, 128]: lin, d01, d02, d12
    gcat = np.zeros((KN, NT, 4, P), dtype=np.float32)
    n = np.arange(NOUT)
    t_i, nl = n // P, n % P
    q0, q1, q2 = q[:, 0], q[:, 1], q[:, 2]
    gcat[q0, t_i, 0, nl] = c0
    gcat[q1, t_i, 0, nl] = c1
    gcat[q2, t_i, 0, nl] = c2
    gcat[q0, t_i, 1, nl] = 1.0
    gcat[q1, t_i, 1, nl] = -1.0
    gcat[q0, t_i, 2, nl] = 1.0
    gcat[q2, t_i, 2, nl] = -1.0
    gcat[q1, t_i, 3, nl] = 1.0
    gcat[q2, t_i, 3, nl] = -1.0
    gcat = np.ascontiguousarray(gcat.reshape(KN, NT * 4 * P)).astype(mmnp)

    # diag: per (tile, plane) diagonal matrices [128,128], coefficient on diag
    acoef = np.stack([a01, a02, a12, au], 0)        # [4, NOUT]
    diag = np.zeros((P, NT, 4, P), dtype=np.float32)
    pp = np.arange(P)
    for t in range(NT):
        for pl in range(4):
            diag[pp, t, pl, pp] = acoef[pl, t * P + pp]
    diag = np.ascontiguousarray(diag.reshape(P, NT * 4 * P)).astype(mmnp)

    # w1 pre-swizzled for lhsT chunks: [128, (k,m) blocks]
    w1t = np.asarray(w1, np.float32).T.reshape(NT, P, 256)       # [k, p, m]
    w1s = np.ascontiguousarray(w1t.transpose(1, 0, 2).reshape(P, NT * 256)).astype(mmnp)
    w2t = np.asarray(w2, np.float32).T.reshape(2, P, P)          # [m, p, o]
    w2s = np.ascontiguousarray(w2t.transpose(1, 0, 2).reshape(P, 2 * P)).astype(mmnp)
    w3s = np.ascontiguousarray(np.asarray(w3, np.float32).T).astype(mmnp)  # [128, 1024]
    b1c = np.ascontiguousarray(np.asarray(b1, np.float32).reshape(2, P).T)
    b2c = np.ascontiguousarray(np.asarray(b2, np.float32).reshape(1, P).T)
    b3c = np.ascontiguousarray(np.asarray(b3, np.float32).reshape(NT, P).T)

    return dict(gcat=gcat, diag=diag, w1s=w1s, w2s=w2s, w3s=w3s,
                b1c=b1c, b2c=b2c, b3c=b3c)


def _build_program(mm_dtype_name="bfloat16"):
    """Build + compile the Bacc program (one NEFF shared by all 8 cores)."""
    key = mm_dtype_name
    if key in _PROG_CACHE:
        return _PROG_CACHE[key]

    import concourse.bacc as bacc
    import concourse.bass as bass
    import concourse.mybir as mybir
    import concourse.tile as tile
    from concourse.masks import make_identity

    f32 = mybir.dt.float32
    mmdt = getattr(mybir.dt, mm_dtype_name)
    AF = mybir.ActivationFunctionType
    ALU = mybir.AluOpType

    nc = bacc.Bacc("TRN2", target_bir_lowering=False, debug=False)

    emb_d = nc.dram_tensor("emb", [S_N, KN], f32, kind="ExternalInput").ap()
    sidx_d = nc.dram_tensor("sidx", [P, NG], mybir.dt.int32, kind="ExternalInput").ap()
    gcat_d = nc.dram_tensor("gcat", [KN, NT * 4 * P], mmdt, kind="ExternalInput").ap()
    diag_d = nc.dram_tensor("diag", [P, NT * 4 * P], mmdt, kind="ExternalInput").ap()
    w1_d = nc.dram_tensor("w1s", [P, NT * 256], mmdt, kind="ExternalInput").ap()
    w2_d = nc.dram_tensor("w2s", [P, 2 * P], mmdt, kind="ExternalInput").ap()
    w3_d = nc.dram_tensor("w3s", [P, NOUT], mmdt, kind="ExternalInput").ap()
    b1_d = nc.dram_tensor("b1c", [P, 2], f32, kind="ExternalInput").ap()
    b2_d = nc.dram_tensor("b2c", [P, 1], f32, kind="ExternalInput").ap()
    b3_d = nc.dram_tensor("b3c", [P, NT], f32, kind="ExternalInput").ap()
    out_d = nc.dram_tensor("out", [NOUT, BL], f32, kind="ExternalOutput").ap()

    def mm(out, lhsT, rhs, start, stop):
        nc.tensor.matmul(out, lhsT, rhs, start=start, stop=stop)

    with tile.TileContext(nc) as tc:
        with (
            tc.tile_pool(name="const", bufs=1) as cpool,
            tc.tile_pool(name="work", bufs=3) as wpool,
            tc.tile_pool(name="pd", bufs=3, space="PSUM") as pd,
            tc.tile_pool(name="pc", bufs=2, space="PSUM") as pc,
            tc.tile_pool(name="pl1", bufs=1, space="PSUM") as pl1,
            tc.tile_pool(name="pmlp", bufs=2, space="PSUM") as pmlp,
        ):
            # ---- student gathers first: they gate the whole Choquet phase ----
            sidx_s = cpool.tile([P, NG], mybir.dt.int32, tag="sidx")
            nc.sync.dma_start(sidx_s[:], sidx_d[:])
            stu_tiles = []
            for g in range(NG):
                stu_g = wpool.tile([P, KN], f32, tag=f"stu{g}")
                nc.gpsimd.indirect_dma_start(
                    out=stu_g[:], out_offset=None, in_=emb_d[:],
                    in_offset=bass.IndirectOffsetOnAxis(ap=sidx_s[:, g:g + 1], axis=0))
                stu_tiles.append(stu_g)

            # ---- PE warm-up burst while DMAs land (HAM un-throttle) ----
            warm = cpool.tile([P, BL], mmdt, tag="warm")
            nc.vector.memset(warm[:], 0.0)
            wps = pd.tile([P, BL], f32, tag="d")
            for _ in range(N_WARM):
                mm(wps, warm[:, :P], warm[:], True, True)

            # ---- constants in (single big DMAs, pre-packed on host) ----
            gcat_s = cpool.tile([KN, NT * 4 * P], mmdt, tag="gcat")
            nc.sync.dma_start(gcat_s[:], gcat_d[:])
            diag_s = cpool.tile([P, NT * 4 * P], mmdt, tag="diag")
            nc.sync.dma_start(diag_s[:], diag_d[:])
            w1_s = cpool.tile([P, NT * 256], mmdt, tag="w1")
            nc.sync.dma_start(w1_s[:], w1_d[:])
            w2_s = cpool.tile([P, 2 * P], mmdt, tag="w2")
            nc.sync.dma_start(w2_s[:], w2_d[:])
            w3_s = cpool.tile([P, NOUT], mmdt, tag="w3")
            nc.sync.dma_start(w3_s[:], w3_d[:])
            b1_s = cpool.tile([P, 2], f32, tag="b1")
            nc.sync.dma_start(b1_s[:], b1_d[:])
            b2_s = cpool.tile([P, 1], f32, tag="b2")
            nc.sync.dma_start(b2_s[:], b2_d[:])
            b3_s = cpool.tile([P, NT], f32, tag="b3")
            nc.sync.dma_start(b3_s[:], b3_d[:])

            ident = cpool.tile([P, P], f32, tag="ident")
            make_identity(nc, ident[:])

            # ---- sigmoid + transpose -> selT [30, 512] ----
            selT = cpool.tile([KN, BL], mmdt, tag="selT")
            for g in range(NG):
                sel_g = wpool.tile([P, KN], f32, tag="sel")
                nc.scalar.activation(sel_g[:], stu_tiles[g][:], AF.Sigmoid)
                tp = pd.tile([KN, P], f32, tag="d")
                nc.tensor.transpose(tp[:], sel_g[:], ident[:])
                nc.vector.tensor_copy(selT[:, g * P:(g + 1) * P], tp[:])

            # ---- phase 1: Choquet integral, tile by tile -> csb [128, 8*512] ----
            csb = cpool.tile([P, NT * BL], mmdt, tag="csb")
            for t in range(NT):
                gbase = t * 4 * P
                bC = pc.tile([P, BL], f32, tag="c")
                bD01 = pd.tile([P, BL], f32, tag="d")
                bD02 = pd.tile([P, BL], f32, tag="d")
                bD12 = pd.tile([P, BL], f32, tag="d")
                mm(bC, gcat_s[:, gbase:gbase + P], selT[:], True, False)
                mm(bD01, gcat_s[:, gbase + P:gbase + 2 * P], selT[:], True, True)
                mm(bD02, gcat_s[:, gbase + 2 * P:gbase + 3 * P], selT[:], True, True)
                mm(bD12, gcat_s[:, gbase + 3 * P:gbase + 4 * P], selT[:], True, True)

                r01 = wpool.tile([P, BL], mmdt, tag="r01")
                nc.scalar.activation(r01[:], bD01[:], AF.Relu)
                r02 = wpool.tile([P, BL], mmdt, tag="r02")
                nc.scalar.activation(r02[:], bD02[:], AF.Relu)
                r12 = wpool.tile([P, BL], mmdt, tag="r12")
                nc.vector.tensor_scalar_max(r12[:], bD12[:], 0.0)
                tD = wpool.tile([P, BL], f32, tag="tD")
                nc.vector.scalar_tensor_tensor(
                    tD[:], bD02[:], 1.0, r01[:], ALU.mult, ALU.subtract)
                u = wpool.tile([P, BL], mmdt, tag="u")
                nc.gpsimd.tensor_scalar_max(u[:], tD[:], 0.0)

                dbase = t * 4 * P
                mm(bC, diag_s[:, dbase:dbase + P], r01[:], False, False)
                mm(bC, diag_s[:, dbase + P:dbase + 2 * P], r02[:], False, False)
                mm(bC, diag_s[:, dbase + 2 * P:dbase + 3 * P], r12[:], False, False)
                mm(bC, diag_s[:, dbase + 3 * P:dbase + 4 * P], u[:], False, True)

                # alternate the PSUM->SBUF eviction between DVE and ACT
                if t % 2 == 0:
                    nc.vector.tensor_copy(csb[:, t * BL:(t + 1) * BL], bC[:])
                else:
                    nc.scalar.copy(csb[:, t * BL:(t + 1) * BL], bC[:])

            # ---- phase 2: MLP ----
            h1 = cpool.tile([P, 2 * BL], mmdt, tag="h1")
            for m in range(2):
                l1p = pl1.tile([P, BL], f32, tag="l1")
                for k in range(NT):
                    mm(l1p, w1_s[:, k * 256 + m * P: k * 256 + (m + 1) * P],
                       csb[:, k * BL:(k + 1) * BL], k == 0, k == NT - 1)
                nc.scalar.activation(h1[:, m * BL:(m + 1) * BL], l1p[:],
                                     AF.Relu, bias=b1_s[:, m:m + 1])

            h2 = cpool.tile([P, BL], mmdt, tag="h2")
            l2p = pmlp.tile([P, BL], f32, tag="l23")
            mm(l2p, w2_s[:, 0:P], h1[:, 0:BL], True, False)
            mm(l2p, w2_s[:, P:2 * P], h1[:, BL:2 * BL], False, True)
            nc.scalar.activation(h2[:], l2p[:], AF.Relu, bias=b2_s[:, 0:1])

            for o in range(NT):
                l3p = pmlp.tile([P, BL], f32, tag="l23")
                mm(l3p, w3_s[:, o * P:(o + 1) * P], h2[:], True, True)
                osb = wpool.tile([P, BL], f32, tag="osb")
                nc.scalar.activation(osb[:], l3p[:], AF.Sigmoid,
                                     bias=b3_s[:, o:o + 1])
                nc.sync.dma_start(out_d[o * P:(o + 1) * P, :], osb[:])

    nc.compile()
    _PROG_CACHE[key] = nc
    return nc


def _run(inputs, trace=False, tmpdir=None, mm_dtype_name="bfloat16"):
    from concourse import bass_utils

    nc = _build_program(mm_dtype_name)

    prep = _host_prep(inputs["q_idx"], inputs["fm_vars"],
                      inputs["w1"], inputs["b1"], inputs["w2"], inputs["b2"],
                      inputs["w3"], inputs["b3"], mm_dtype_name)
    emb = np.ascontiguousarray(np.asarray(inputs["emb"], np.float32))
    stu_id = np.asarray(inputs["stu_id"]).astype(np.int32)

    in_maps = []
    for c in range(NCORES):
        sidx = np.ascontiguousarray(
            stu_id[c * BL:(c + 1) * BL].reshape(NG, P).T).astype(np.int32)
        in_maps.append(dict(emb=emb, sidx=sidx, **prep))

    if trace:
        # NTFF profiling needs the antenv.axon_hooks shim + no S3 upload.
        import sys, types
        if "antenv.axon_hooks" not in sys.modules:
            import trn_agent_boot.trn_boot as tb
            mod = types.ModuleType("antenv.axon_hooks")
            hook = tb._ntff_profile_via_ctypes("/opt/axon/libaxon_pjrt.so")
            mod.get_axon_ntff_profile_hook = lambda: hook
            mod.set_axon_ntff_profile_hook = lambda h: None
            sys.modules["antenv.axon_hooks"] = mod
        bass_utils.upload_artifacts = lambda d: d

    res = bass_utils.run_bass_kernel_spmd(
        nc, in_maps, core_ids=list(range(NCORES)), trace=trace, tmpdir=tmpdir)

    out = np.concatenate(
        [res.results[c]["out"].T for c in range(NCORES)], axis=0)
    return np.ascontiguousarray(out.astype(np.float32)), res


def kernel(**inputs):
    out, _ = _run(inputs, trace=False)
    return out


# revision 8
# speedup vs baseline: 1.6211x; 1.6211x over previous
"""Trainium2 Bass kernel for nn_CICDM (Choquet-integral cognitive-diagnosis model).

Computation (see reference):
  sel = sigmoid(emb[stu_id])                       # [B, 30]
  x_k = sel[:, q_idx[:, k]]  k=0,1,2               # [B, N]
  C   = Choquet integral of (x0,x1,x2) against fuzzy measure FM(fm_vars)
  out = sigmoid(relu(relu(C@w1.T+b1)@w2.T+b2)@w3.T+b3)

Key reformulation: the sorted-difference Choquet integral equals its Mobius
form  C = sum_S m(S) * min_{i in S} x_i  which, with hinge algebra
(min(a,b) = a - relu(a-b)), becomes

  C = c0*x0 + c1*x1 + c2*x2 + a01*r01 + a02*r02 + a12*r12 + au*u
  r01 = relu(x0-x1), r02 = relu(x0-x2), r12 = relu(x1-x2), u = relu((x0-x2)-r01)

with per-exercise constants c*/a* derived from fm_vars on the host.  The
per-exercise gathers x_k and differences are one-hot / +-1 matmuls on the
tensor engine (K=30), the per-exercise scaling is a diagonal-matmul
accumulation into PSUM, and the hinges run on the scalar/vector/gpsimd
engines.

Distribution: data-parallel over the batch: 8 cores x 512 rows. Everything is
computed in transposed layout (exercises on partitions, batch on free dim) so
per-exercise coefficients are per-partition scalars.
"""

import numpy as np

B = 4096
NCORES = 8
BL = B // NCORES          # 512 local batch
KN = 30
NOUT = 1024
NT = NOUT // 128          # 8 exercise tiles
P = 128
NG = BL // P              # 4 gather groups per core
S_N = 100000
N_WARM = 18               # PE warm-up matmuls issued during the DMA phase

_PROG_CACHE = {}


def _np_dt(name):
    if name == "bfloat16":
        import ml_dtypes
        return np.dtype(ml_dtypes.bfloat16)
    return np.dtype(np.float32)


def _host_prep(q_idx, fm_vars, w1, b1, w2, b2, w3, b3, mm_dtype_name):
    """Derive all per-exercise constants + weight layouts on the host."""
    mmnp = _np_dt(mm_dtype_name)
    q = np.asarray(q_idx).astype(np.int64)          # [N, 3]
    fm = np.asarray(fm_vars, dtype=np.float32)

    chi = np.abs(fm)
    f0, f1 = chi[0], chi[1]
    f2 = np.maximum(f0, f1) + chi[2]
    f3 = chi[3]
    f4 = np.maximum(f3, f0) + chi[4]
    f5 = np.maximum(f3, f1) + chi[5]
    FM = np.minimum(np.stack([f0, f1, f2, f3, f4, f5, np.ones_like(f0)], 0), 1.0)
    F0, F1, F2, F3, F4, F5, F6 = FM.astype(np.float64)
    m0, m1, m3 = F0, F1, F3
    m2 = F2 - F0 - F1
    m4 = F4 - F0 - F3
    m5 = F5 - F1 - F3
    m6 = F6 - F2 - F4 - F5 + F0 + F1 + F3
    c0 = (m0 + m2 + m4).astype(np.float32)
    c1 = (m1 + m5).astype(np.float32)
    c2 = (m3 + m6).astype(np.float32)
    a01 = (-(m2 + m6)).astype(np.float32)
    a02 = (-m4).astype(np.float32)
    a12 = (-m5).astype(np.float32)
    au = m6.astype(np.float32)

    # gcat: per tile t, 4 lhsT planes [30, 128]: lin, d01, d02, d12
    gcat = np.zeros((KN, NT, 4, P), dtype=np.float32)
    n = np.arange(NOUT)
    t_i, nl = n // P, n % P
    q0, q1, q2 = q[:, 0], q[:, 1], q[:, 2]
    gcat[q0, t_i, 0, nl] = c0
    gcat[q1, t_i, 0, nl] = c1
    gcat[q2, t_i, 0, nl] = c2
    gcat[q0, t_i, 1, nl] = 1.0
    gcat[q1, t_i, 1, nl] = -1.0
    gcat[q0, t_i, 2, nl] = 1.0
    gcat[q2, t_i, 2, nl] = -1.0
    gcat[q1, t_i, 3, nl] = 1.0
    gcat[q2, t_i, 3, nl] = -1.0
    gcat = np.ascontiguousarray(gcat.reshape(KN, NT * 4 * P)).astype(mmnp)

    # diag: per (tile, plane) diagonal matrices [128,128], coefficient on diag
    acoef = np.stack([a01, a02, a12, au], 0)        # [4, NOUT]
    diag = np.zeros((P, NT, 4, P), dtype=np.float32)
    pp = np.arange(P)
    for t in range(NT):
        for pl in range(4):
            diag[pp, t, pl, pp] = acoef[pl, t * P + pp]
    diag = np.ascontiguousarray(diag.reshape(P, NT * 4 * P)).astype(mmnp)

    # w1 pre-swizzled for lhsT chunks: [128, (k,m) blocks]
    w1t = np.asarray(w1, np.float32).T.reshape(NT, P, 256)       # [k, p, m]
    w1s = np.ascontiguousarray(w1t.transpose(1, 0, 2).reshape(P, NT * 256)).astype(mmnp)
    w2t = np.asarray(w2, np.float32).T.reshape(2, P, P)          # [m, p, o]
    w2s = np.ascontiguousarray(w2t.transpose(1, 0, 2).reshape(P, 2 * P)).astype(mmnp)
    w3s = np.ascontiguousarray(np.asarray(w3, np.float32).T).astype(mmnp)  # [128, 1024]
    b1c = np.ascontiguousarray(np.asarray(b1, np.float32).reshape(2, P).T)
    b2c = np.ascontiguousarray(np.asarray(b2, np.float32).reshape(1, P).T)
    b3c = np.ascontiguousarray(np.asarray(b3, np.float32).reshape(NT, P).T)

    return dict(gcat=gcat, diag=diag, w1s=w1s, w2s=w2s, w3s=w3s,
                b1c=b1c, b2c=b2c, b3c=b3c)


def _build_program(mm_dtype_name="bfloat16"):
    """Build + compile the Bacc program (one NEFF shared by all 8 cores)."""
    key = mm_dtype_name
    if key in _PROG_CACHE:
        return _PROG_CACHE[key]

    import concourse.bacc as bacc
    import concourse.bass as bass
    import concourse.mybir as mybir
    import concourse.tile as tile
    from concourse.masks import make_identity

    f32 = mybir.dt.float32
    mmdt = getattr(mybir.dt, mm_dtype_name)
    AF = mybir.ActivationFunctionType
    ALU = mybir.AluOpType

    nc = bacc.Bacc("TRN2", target_bir_lowering=False, debug=False,
                   num_swdge_queues=4)

    emb_d = nc.dram_tensor("emb", [S_N, KN], f32, kind="ExternalInput").ap()
    sidx_d = nc.dram_tensor("sidx", [P, NG], mybir.dt.int32, kind="ExternalInput").ap()
    gcat_d = nc.dram_tensor("gcat", [KN, NT * 4 * P], mmdt, kind="ExternalInput").ap()
    diag_d = nc.dram_tensor("diag", [P, NT * 4 * P], mmdt, kind="ExternalInput").ap()
    w1_d = nc.dram_tensor("w1s", [P, NT * 256], mmdt, kind="ExternalInput").ap()
    w2_d = nc.dram_tensor("w2s", [P, 2 * P], mmdt, kind="ExternalInput").ap()
    w3_d = nc.dram_tensor("w3s", [P, NOUT], mmdt, kind="ExternalInput").ap()
    b1_d = nc.dram_tensor("b1c", [P, 2], f32, kind="ExternalInput").ap()
    b2_d = nc.dram_tensor("b2c", [P, 1], f32, kind="ExternalInput").ap()
    b3_d = nc.dram_tensor("b3c", [P, NT], f32, kind="ExternalInput").ap()
    out_d = nc.dram_tensor("out", [NOUT, BL], f32, kind="ExternalOutput").ap()

    def mm(out, lhsT, rhs, start, stop):
        nc.tensor.matmul(out, lhsT, rhs, start=start, stop=stop)

    with tile.TileContext(nc) as tc:
        with (
            tc.tile_pool(name="const", bufs=1) as cpool,
            tc.tile_pool(name="work", bufs=3) as wpool,
            tc.tile_pool(name="pd", bufs=3, space="PSUM") as pd,
            tc.tile_pool(name="pc", bufs=1, space="PSUM") as pc,
            tc.tile_pool(name="pl1", bufs=2, space="PSUM") as pl1,
            tc.tile_pool(name="pmlp", bufs=2, space="PSUM") as pmlp,
        ):
            # ---- student gathers first: they gate the whole Choquet phase ----
            sidx_s = cpool.tile([P, NG], mybir.dt.int32, tag="sidx")
            nc.gpsimd.dma_start(sidx_s[:], sidx_d[:])
            stu_tiles = []
            for g in range(NG):
                stu_g = wpool.tile([P, KN], f32, tag=f"stu{g}")
                nc.gpsimd.indirect_dma_start(
                    out=stu_g[:], out_offset=None, in_=emb_d[:],
                    in_offset=bass.IndirectOffsetOnAxis(ap=sidx_s[:, g:g + 1], axis=0))
                stu_tiles.append(stu_g)

            # ---- PE warm-up burst while DMAs land (HAM un-throttle) ----
            warm = cpool.tile([P, BL], mmdt, tag="warm")
            nc.vector.memset(warm[:], 0.0)
            wps = pd.tile([P, BL], f32, tag="d")
            for _ in range(N_WARM):
                mm(wps, warm[:, :P], warm[:], True, True)

            # ---- constants in (single big DMAs, pre-packed on host) ----
            gcat_s = cpool.tile([KN, NT * 4 * P], mmdt, tag="gcat")
            nc.sync.dma_start(gcat_s[:], gcat_d[:])
            diag_s = cpool.tile([P, NT * 4 * P], mmdt, tag="diag")
            nc.sync.dma_start(diag_s[:], diag_d[:])
            w1_s = cpool.tile([P, NT * 256], mmdt, tag="w1")
            nc.sync.dma_start(w1_s[:], w1_d[:])
            w2_s = cpool.tile([P, 2 * P], mmdt, tag="w2")
            nc.sync.dma_start(w2_s[:], w2_d[:])
            w3_s = cpool.tile([P, NOUT], mmdt, tag="w3")
            nc.sync.dma_start(w3_s[:], w3_d[:])
            b1_s = cpool.tile([P, 2], f32, tag="b1")
            nc.sync.dma_start(b1_s[:], b1_d[:])
            b2_s = cpool.tile([P, 1], f32, tag="b2")
            nc.sync.dma_start(b2_s[:], b2_d[:])
            b3_s = cpool.tile([P, NT], f32, tag="b3")
            nc.sync.dma_start(b3_s[:], b3_d[:])

            ident = cpool.tile([P, P], f32, tag="ident")
            make_identity(nc, ident[:])

            # ---- sigmoid + transpose -> selT [30, 512] ----
            selT = cpool.tile([KN, BL], mmdt, tag="selT")
            for g in range(NG):
                sel_g = wpool.tile([P, KN], f32, tag="sel")
                nc.scalar.activation(sel_g[:], stu_tiles[g][:], AF.Sigmoid)
                tp = pd.tile([KN, P], f32, tag="d")
                nc.tensor.transpose(tp[:], sel_g[:], ident[:])
                nc.vector.tensor_copy(selT[:, g * P:(g + 1) * P], tp[:])

            # ---- phase 1: Choquet integral, tile by tile -> csb [128, 8*512] ----
            csb = cpool.tile([P, NT * BL], mmdt, tag="csb")
            l1ps = [pl1.tile([P, BL], f32, tag="l1", name=f"l1p{m}") for m in range(2)]
            for t in range(NT):
                gbase = t * 4 * P
                bC = pc.tile([P, BL], f32, tag="c")
                bD01 = pd.tile([P, BL], f32, tag="d")
                bD02 = pd.tile([P, BL], f32, tag="d")
                bD12 = pd.tile([P, BL], f32, tag="d")
                mm(bC, gcat_s[:, gbase:gbase + P], selT[:], True, False)
                mm(bD01, gcat_s[:, gbase + P:gbase + 2 * P], selT[:], True, True)
                mm(bD02, gcat_s[:, gbase + 2 * P:gbase + 3 * P], selT[:], True, True)
                mm(bD12, gcat_s[:, gbase + 3 * P:gbase + 4 * P], selT[:], True, True)

                r01 = wpool.tile([P, BL], mmdt, tag="r01")
                nc.scalar.activation(r01[:], bD01[:], AF.Relu)
                r02 = wpool.tile([P, BL], mmdt, tag="r02")
                nc.scalar.activation(r02[:], bD02[:], AF.Relu)
                r12 = wpool.tile([P, BL], mmdt, tag="r12")
                nc.vector.tensor_scalar_max(r12[:], bD12[:], 0.0)
                wmin = wpool.tile([P, BL], mmdt, tag="wmin")
                nc.vector.scalar_tensor_tensor(
                    wmin[:], bD02[:], 1.0, r01[:], ALU.mult, ALU.min)

                dbase = t * 4 * P
                mm(bC, diag_s[:, dbase:dbase + P], r01[:], False, False)
                mm(bC, diag_s[:, dbase + P:dbase + 2 * P], r02[:], False, False)
                mm(bC, diag_s[:, dbase + 2 * P:dbase + 3 * P], r12[:], False, False)
                mm(bC, diag_s[:, dbase + 3 * P:dbase + 4 * P], wmin[:], False, True)

                nc.vector.tensor_copy(csb[:, t * BL:(t + 1) * BL], bC[:])
                # layer-1 matmul chunks interleave with the Choquet tiles
                for m in range(2):
                    mm(l1ps[m], w1_s[:, t * 256 + m * P: t * 256 + (m + 1) * P],
                       csb[:, t * BL:(t + 1) * BL], t == 0, t == NT - 1)

            # ---- phase 2: MLP ----
            h1 = cpool.tile([P, 2 * BL], mmdt, tag="h1")
            for m in range(2):
                nc.scalar.activation(h1[:, m * BL:(m + 1) * BL], l1ps[m][:],
                                     AF.Relu, bias=b1_s[:, m:m + 1])

            h2 = cpool.tile([P, BL], mmdt, tag="h2")
            l2p = pmlp.tile([P, BL], f32, tag="l23")
            mm(l2p, w2_s[:, 0:P], h1[:, 0:BL], True, False)
            mm(l2p, w2_s[:, P:2 * P], h1[:, BL:2 * BL], False, True)
            nc.scalar.activation(h2[:], l2p[:], AF.Relu, bias=b2_s[:, 0:1])

            for o in range(NT):
                l3p = pmlp.tile([P, BL], f32, tag="l23")
                mm(l3p, w3_s[:, o * P:(o + 1) * P], h2[:], True, True)
                osb = wpool.tile([P, BL], f32, tag="osb")
                nc.scalar.activation(osb[:], l3p[:], AF.Sigmoid,
                                     bias=b3_s[:, o:o + 1])
                nc.sync.dma_start(out_d[o * P:(o + 1) * P, :], osb[:])

    nc.compile()
    _PROG_CACHE[key] = nc
    return nc


def _run(inputs, trace=False, tmpdir=None, mm_dtype_name="bfloat16"):
    from concourse import bass_utils

    nc = _build_program(mm_dtype_name)

    prep = _host_prep(inputs["q_idx"], inputs["fm_vars"],
                      inputs["w1"], inputs["b1"], inputs["w2"], inputs["b2"],
                      inputs["w3"], inputs["b3"], mm_dtype_name)
    emb = np.ascontiguousarray(np.asarray(inputs["emb"], np.float32))
    stu_id = np.asarray(inputs["stu_id"]).astype(np.int32)

    in_maps = []
    for c in range(NCORES):
        sidx = np.ascontiguousarray(
            stu_id[c * BL:(c + 1) * BL].reshape(NG, P).T).astype(np.int32)
        in_maps.append(dict(emb=emb, sidx=sidx, **prep))

    if trace:
        # NTFF profiling needs the antenv.axon_hooks shim + no S3 upload.
        import sys, types
        if "antenv.axon_hooks" not in sys.modules:
            import trn_agent_boot.trn_boot as tb
            mod = types.ModuleType("antenv.axon_hooks")
            hook = tb._ntff_profile_via_ctypes("/opt/axon/libaxon_pjrt.so")
            mod.get_axon_ntff_profile_hook = lambda: hook
            mod.set_axon_ntff_profile_hook = lambda h: None
            sys.modules["antenv.axon_hooks"] = mod
        bass_utils.upload_artifacts = lambda d: d

    res = bass_utils.run_bass_kernel_spmd(
        nc, in_maps, core_ids=list(range(NCORES)), trace=trace, tmpdir=tmpdir)

    out = np.concatenate(
        [res.results[c]["out"].T for c in range(NCORES)], axis=0)
    return np.ascontiguousarray(out.astype(np.float32)), res


def kernel(**inputs):
    out, _ = _run(inputs, trace=False)
    return out
